# revision 49
# baseline (speedup 1.0000x reference)
"""Trainium2 Bass kernel for a local-attention layer (GQA + RoPE + banded mask).

Full computation (reference semantics, f32):
  q = x@wq, k = x@wk, v = x@wv  (B=2, S=2048, D=2048, Hq=16, Hkv=4, hd=128)
  rope(q), rope(k) interleaved-pair style
  banded causal attention, window=1024, softmax
  out = (probs @ v_rep) @ wo

Sharding: 8 cores = (batch b in {0,1}) x (kv-group g in {0..3}).
Core c handles batch c//4, kv head c%4 and its 4 q heads.  Each core
computes a partial (2048, 2048) f16 output (its heads' contribution
through wo rows); host sums the 4 partials per batch in f32.

Per-core kernel strategy (all matmuls fp16, f32 PSUM accum):
  - X^T materialized in SBUF by XBAR DMA-transpose straight from DRAM
    (x shipped as f16 from host) -- no PE transposes, no PSUM staging.
  - Projections computed transposed one head at a time: QT/KT/VT =
    W^T X^T from natural W layout (lhsT=W chunk, rhs=XT chunk).
  - RoPE in "half-split" form on f16 SBUF tiles (host pre-permutes
    wq/wk columns; 1/sqrt(hd) folded into wq): y = x*c2 + swap(x)*s2n
    with duplicated cos and sign-folded sin so every two-input DVE op
    keeps both inputs at base partition 0 (HW verifier rule).
  - V transposed to natural (s, d) tiles by SBUF->SBUF DMA-transpose.
  - Scores transposed ST[j,i] = kt_tile^T @ QT, banded tight: per j-tile
    only the i-subtiles inside the band are computed; exp on ScalarE per
    written window.  Post-exp masks (gpsimd.affine_select, fill 0) only
    on the diagonal and window-edge 128x128 blocks.
  - pv/dn PSUM banks explicitly zeroed (ACT memzero / DVE memset), all
    accumulating matmuls start=False with skip_group_check so ragged
    band windows can accumulate in pipeline order.
  - Softmax denominator via ones[128,128] stationary matmul -> PSUM is
    the broadcast denominator; reciprocal on DVE; normalization fused
    into the PSUM->SBUF copy of the attention output.
  - o-proj accumulates in PSUM, staged to f16 SBUF (ACT/DVE alternating)
    and DMA'd out as f16; host sums the 4 partials per batch.
  - Software pipelining: chunk c's attention/o-proj interleaved with
    chunk c+1's projections; PSUM statically fits 8 banks; startup DMAs
    ordered just-in-time across the SWDGE and two HWDGE queues.
"""

import os
import numpy as np

B, S, D = 2, 2048, 2048
NH, NKV, HD = 16, 4, 128
WINDOW = 1024
ROPE_THETA = 10000.0
HQ = NH // NKV          # q heads per core = 4
QD = HQ * HD            # 512
NK = D // 128           # 16 contraction chunks
CH = 512                # s-chunk size
NCH = S // CH           # 4 chunks
NSUB = CH // 128        # 4 s-subtiles per chunk

_cache = {}


def _host_prep(wq, wk, wv, wo):
    """Per-core f16 weight slices with rope permutation + scale folded in."""
    # de-interleave permutation: dev col j <- ref col (2j if j<64 else 2(j-64)+1)
    perm = np.empty(HD, dtype=np.int64)
    perm[:64] = np.arange(64) * 2
    perm[64:] = np.arange(64) * 2 + 1

    scale = 1.0 / np.sqrt(np.float32(HD))
    wq_p = (wq.reshape(D, NH, HD)[:, :, perm].reshape(D, NH * HD) * scale)
    wk_p = wk.reshape(D, NKV, HD)[:, :, perm].reshape(D, NKV * HD)

    inv_freq = 1.0 / (ROPE_THETA ** (np.arange(0, HD, 2, dtype=np.float32) / HD))
    t = np.arange(S, dtype=np.float32)
    ang = np.outer(t, inv_freq)             # (S, 64)
    cosT = np.cos(ang).T.astype(np.float16)  # (64, S)
    sinT = np.sin(ang).T.astype(np.float16)
    # duplicated cos; sign-folded sin: y = x*c2 + swap(x)*s2n, all ops
    # partition-base aligned (HW: two SBUF inputs must share base partition)
    c2 = np.ascontiguousarray(np.concatenate([cosT, cosT], axis=0))    # (128,S)
    s2n = np.ascontiguousarray(np.concatenate([-sinT, sinT], axis=0))

    shards = []
    for c in range(8):
        g = c % 4
        wkv = np.concatenate(
            [wk_p[:, g * HD:(g + 1) * HD], wv[:, g * HD:(g + 1) * HD]], axis=1)
        sh = dict(
            wkv=np.ascontiguousarray(wkv).astype(np.float16),
            wq01=np.ascontiguousarray(
                wq_p[:, g * QD:g * QD + 2 * HD]).astype(np.float16),
            wq23=np.ascontiguousarray(
                wq_p[:, g * QD + 2 * HD:(g + 1) * QD]).astype(np.float16),
            wo=np.ascontiguousarray(wo[g * QD:(g + 1) * QD, :]).astype(np.float16),
        )
        shards.append(sh)
    return shards, c2, s2n


def _jt_win(c, jt):
    """Allowed i-subtile window [lo, hi] (global isub idx) for j-tile jt in chunk c."""
    lo = max(4 * c, jt)
    hi = min(4 * c + 3, jt + WINDOW // 128)
    return lo, hi


def build_kernel():
    import concourse.bass as bass
    import concourse.mybir as mybir
    import concourse.tile as tile
    from concourse import bacc

    f16 = mybir.dt.float16
    f32 = mybir.dt.float32
    EXP = mybir.ActivationFunctionType.Exp
    GE = mybir.AluOpType.is_ge
    NW = WINDOW // 128      # 8

    nc = bacc.Bacc("TRN2", target_bir_lowering=False, debug=False, num_devices=8)

    x16_d = nc.dram_tensor("x16", [S, D], f16, kind="ExternalInput").ap()
    wkv_d = nc.dram_tensor("wkv", [D, 2 * HD], f16, kind="ExternalInput").ap()
    wq01_d = nc.dram_tensor("wq01", [D, 2 * HD], f16, kind="ExternalInput").ap()
    wq23_d = nc.dram_tensor("wq23", [D, 2 * HD], f16, kind="ExternalInput").ap()
    wo_d = nc.dram_tensor("wo", [QD, D], f16, kind="ExternalInput").ap()
    c_d = nc.dram_tensor("c2", [128, S], f16, kind="ExternalInput").ap()
    s_d = nc.dram_tensor("s2n", [128, S], f16, kind="ExternalInput").ap()
    one_d = nc.dram_tensor("ones", [128, 128], f16, kind="ExternalInput").ap()
    out_d = nc.dram_tensor("out", [S, D], f16, kind="ExternalOutput").ap()

    with tile.TileContext(nc) as tc:
        with (
            tc.tile_pool(name="persist", bufs=1) as pp,
            tc.tile_pool(name="xtpool", bufs=2) as xtp,
            tc.tile_pool(name="qtpool", bufs=8) as qtp,
            tc.tile_pool(name="ropet", bufs=2) as rtp,
            tc.tile_pool(name="ropeab", bufs=4) as rab,
            tc.tile_pool(name="vtpool", bufs=2) as vtp,
            tc.tile_pool(name="ptpool", bufs=8) as ptp,
            tc.tile_pool(name="atpool", bufs=6) as atp,
            tc.tile_pool(name="rcpool", bufs=2) as rcp_p,
            tc.tile_pool(name="obpool", bufs=3) as obp,
            tc.tile_pool(name="psProj", bufs=2, space="PSUM") as pjp,
            tc.tile_pool(name="psBig", bufs=2, space="PSUM") as stp,
            tc.tile_pool(name="psPV", bufs=1, space="PSUM") as pvp,
            tc.tile_pool(name="psDN", bufs=1, space="PSUM") as dnp,
        ):
            # ---- persistent SBUF tensors -------------------------------
            wkv_sb = pp.tile([128, NK * 2 * HD], f16, tag="wkv")  # [k][wk|wv]
            wq_sb = pp.tile([128, NK * QD], f16, tag="wq")        # [k][:, qd]
            wo_sb = pp.tile([128, HQ * D], f16, tag="wo")       # [h][:, e]
            c_sb = pp.tile([128, S], f16, tag="c")
            s_sb = pp.tile([128, S], f16, tag="s")
            one_sb = pp.tile([128, 128], f16, tag="one")
            kt_sb = pp.tile([128, S], f16, tag="kt")            # rope'd K^T
            v_sb = pp.tile([128, NK * 128], f16, tag="v")       # [jt][s, d]

            xts = [None] * NCH

            def issue_xt(c, nsub=2):
                """XBAR DMA-transpose X^T for chunk c straight from DRAM,
                split so early k-tiles land sooner."""
                xt = xtp.tile([128, NK * CH], f16, tag="xt", name=f"xt{c}")
                kn = NK // nsub
                for q in range(nsub):
                    nc.sync.dma_start_transpose(
                        xt[:, kn * q * CH:kn * (q + 1) * CH]
                        .rearrange("p (k s) -> p k s", k=kn),
                        x16_d[c * CH:(c + 1) * CH,
                              kn * q * 128:kn * (q + 1) * 128])
                xts[c] = xt

            # startup DMA order matters: the DMA transfers serialize on a
            # shared device in arrival order, so issue just-in-time for
            # the chunk-0 pipeline (interleaved with the proj passes below).
            def dma_startup_head():
                # wkv via SWDGE; x^T(0) halves split across the two HWDGE
                # queues so the second half pre-queues during the first
                nc.gpsimd.dma_start(
                    wkv_sb[:].rearrange("p (k n) -> p k n", k=NK),
                    wkv_d.rearrange("(k p) n -> p k n", p=128))
                xt = xtp.tile([128, NK * CH], f16, tag="xt", name="xt0")
                for q in range(4):
                    eng = nc.sync if q % 2 == 0 else nc.scalar
                    eng.dma_start_transpose(
                        xt[:, 4 * q * CH:4 * (q + 1) * CH]
                        .rearrange("p (k s) -> p k s", k=4),
                        x16_d[0:CH, 4 * q * 128:4 * (q + 1) * 128])
                xts[0] = xt
                # trig + ones via SWDGE: lands early, in parallel with the
                # HWDGE stream (the rope ops head the DVE queue and would
                # otherwise stall all DVE retirement on these)
                nc.gpsimd.dma_start(c_sb[:], c_d)
                nc.gpsimd.dma_start(s_sb[:], s_d)
                nc.gpsimd.dma_start(one_sb[:], one_d)
                nc.scalar.dma_start(
                    wq_sb[:, 0:NK * 2 * HD]
                    .rearrange("p (k n) -> p k n", k=NK),
                    wq01_d.rearrange("(k p) n -> p k n", p=128))
                nc.scalar.dma_start(
                    wq_sb[:, NK * 2 * HD:]
                    .rearrange("p (k n) -> p k n", k=NK),
                    wq23_d.rearrange("(k p) n -> p k n", p=128))

            def rope(dst, src, c):
                """src (128, CH) f16 sbuf -> dst (128, CH) f16, half-split rope:
                dst = src*c2 + swap(src)*s2n.  Two-input DVE ops keep both
                inputs at base partition 0 (HW verifier requirement)."""
                cs = c_sb[:, c * CH:(c + 1) * CH]
                sn = s_sb[:, c * CH:(c + 1) * CH]
                sw = rab.tile([128, CH], f16, tag="ra")
                nc.vector.tensor_copy(sw[0:64, :], src[64:128, :])
                nc.vector.tensor_copy(sw[64:128, :], src[0:64, :])
                a = rab.tile([128, CH], f16, tag="ra")
                nc.vector.tensor_mul(a[:], src[:], cs)
                nc.vector.tensor_mul(sw[:], sw[:], sn)
                nc.vector.tensor_add(dst[:], a[:], sw[:])

            def proj_pass_off(c, w_sb, off, stride):
                """One 128-wide projection: returns f16 copy in SBUF."""
                ps = pjp.tile([128, CH], f32, tag="pj", name=f"pj{c}_{off}")
                xt = xts[c]
                for k in range(NK):
                    nc.tensor.matmul(
                        ps[:],
                        w_sb[:, k * stride + off: k * stride + off + 128],
                        xt[:, k * CH:(k + 1) * CH],
                        start=(k == 0), stop=(k == NK - 1),
                    )
                xf = rtp.tile([128, CH], f16, tag="xf", name=f"xf{c}_{off}")
                nc.vector.tensor_copy(xf[:], ps[:])
                return xf

            proj_pass = proj_pass_off

            def emit_proj(c, which):
                """which in {'k','v','q0'..'q3'}"""
                if which == "k":
                    xf = proj_pass_off(c, wkv_sb, 0, 2 * HD)
                    rope(kt_sb[:, c * CH:(c + 1) * CH], xf, c)
                elif which == "v":
                    ps = pjp.tile([128, CH], f32, tag="pj", name=f"pjv{c}")
                    xt = xts[c]
                    for k in range(NK):
                        nc.tensor.matmul(
                            ps[:], wkv_sb[:, k * 2 * HD + HD: (k + 1) * 2 * HD],
                            xt[:, k * CH:(k + 1) * CH],
                            start=(k == 0), stop=(k == NK - 1),
                        )
                    vt = vtp.tile([128, CH], f16, tag="vt", name=f"vt{c}")
                    nc.scalar.copy(vt[:], ps[:])
                    # natural (s, d) tiles via SBUF->SBUF DMA transpose
                    nc.sync.dma_start_transpose(
                        v_sb[:, c * NSUB * 128:(c + 1) * NSUB * 128]
                        .rearrange("p (g d) -> p g d", g=NSUB),
                        vt[:])
                else:
                    h = int(which[1])
                    off = (h // 2) * NK * 2 * HD
                    xf = proj_pass_off(c, wq_sb, off + (h % 2) * HD, 2 * HD)
                    qt = qtp.tile([128, CH], f16, tag="qt", name=f"qt{c}_{h}")
                    rope(qt, xf, c)
                    qts[(c, h)] = qt

            qts = {}
            ats = {}

            def emit_att_head(c, h, alt_pool=False):
                jt0 = max(0, 4 * c - NW)
                jts = list(range(jt0, 4 * c + 4))
                # tail chunk: odd heads borrow the (idle) projection PSUM
                # pool so head boundaries double-buffer pv/dn
                pool_v = pjp if alt_pool else pvp
                pool_d = pjp if alt_pool else dnp
                pv_ps = pool_v.tile([128, CH], f32, tag="pj" if alt_pool else "pv",
                                    name=f"pv{c}_{h}")
                dn_ps = pool_d.tile([128, CH], f32, tag="pj" if alt_pool else "dn",
                                    name=f"dn{c}_{h}")
                # explicit zero + start=False accumulation everywhere: lets
                # ragged-window matmuls accumulate in natural (pipeline) order
                nc.scalar.memzero(pv_ps[:])
                nc.vector.memset(dn_ps[:], 0.0)
                pts = []
                for pi in range(0, len(jts), 2):
                    pair = jts[pi:pi + 2]
                    st = stp.tile([128, 2 * CH], f32, tag="big", name=f"st{c}_{h}_{pi}")
                    for q_, jt in enumerate(pair):
                        lo, hi = _jt_win(c, jt)
                        a0 = (lo - 4 * c) * 128
                        a1 = (hi - 4 * c + 1) * 128
                        nc.tensor.matmul(
                            st[:, q_ * CH + a0: q_ * CH + a1],
                            kt_sb[:, jt * 128:(jt + 1) * 128],
                            qts[(c, h)][:, a0:a1],
                            start=True, stop=True,
                        )
                    pt = ptp.tile([128, 2 * CH], f16, tag="pt", name=f"pt{c}_{h}_{pi}")
                    for q_, jt in enumerate(pair):
                        lo, hi = _jt_win(c, jt)
                        a0 = q_ * CH + (lo - 4 * c) * 128
                        a1 = q_ * CH + (hi - 4 * c + 1) * 128
                        nc.scalar.activation(pt[:, a0:a1], st[:, a0:a1], EXP)
                    for q_, jt in enumerate(pair):
                        if 4 * c <= jt:          # diagonal block: keep j <= i
                            o = q_ * CH + (jt - 4 * c) * 128
                            nc.gpsimd.affine_select(
                                out=pt[:, o:o + 128], in_=pt[:, o:o + 128],
                                pattern=[[1, 128]], compare_op=GE, fill=0.0,
                                base=0, channel_multiplier=-1,
                            )
                        if jt + NW <= 4 * c + 3:  # window edge: keep j > i-W-1
                            o = q_ * CH + (jt + NW - 4 * c) * 128
                            nc.gpsimd.affine_select(
                                out=pt[:, o:o + 128], in_=pt[:, o:o + 128],
                                pattern=[[-1, 128]], compare_op=GE, fill=0.0,
                                base=0, channel_multiplier=1,
                            )
                    pts.append((pair, pt))

                items = []
                for pi, (pair, pt) in enumerate(pts):
                    for q_, jt in enumerate(pair):
                        lo, hi = _jt_win(c, jt)
                        items.append((jt, (lo - 4 * c) * 128,
                                      (hi - 4 * c + 1) * 128, q_, pt))
                for n_, (jt, a0, a1, q_, pt) in enumerate(items):
                    nc.tensor.matmul(
                        pv_ps[:, a0:a1],
                        v_sb[:, jt * 128:(jt + 1) * 128],
                        pt[:, q_ * CH + a0: q_ * CH + a1],
                        start=False, stop=(n_ == len(items) - 1),
                        skip_group_check=True,
                    )
                for n_, (jt, a0, a1, q_, pt) in enumerate(items):
                    nc.tensor.matmul(
                        dn_ps[:, a0:a1],
                        one_sb[:],
                        pt[:, q_ * CH + a0: q_ * CH + a1],
                        start=False, stop=(n_ == len(items) - 1),
                        skip_group_check=True,
                    )
                rcp = rcp_p.tile([128, CH], f32, tag="rcp", name=f"rcp{c}_{h}")
                nc.vector.reciprocal(rcp[:], dn_ps[:])
                at = atp.tile([128, CH], f16, tag="at", name=f"at{c}_{h}")
                nc.vector.tensor_mul(at[:], pv_ps[:], rcp[:])
                ats[(c, h)] = at

            def emit_oproj(c):
                for g in range(NSUB):
                    for half in range(2):
                        op_ = stp.tile([128, 2 * CH], f32, tag="big",
                                       name=f"op{c}_{g}_{half}")
                        for e2 in range(2):
                            ecol = half * 1024 + e2 * CH
                            for h in range(HQ):
                                nc.tensor.matmul(
                                    op_[:, e2 * CH:(e2 + 1) * CH],
                                    ats[(c, h)][:, g * 128:(g + 1) * 128],
                                    wo_sb[:, h * D + ecol: h * D + ecol + CH],
                                    start=(h == 0), stop=(h == HQ - 1),
                                )
                        ob = obp.tile([128, 2 * CH], f16, tag="ob",
                                      name=f"ob{c}_{g}_{half}")
                        if half == 0:
                            nc.scalar.copy(ob[:], op_[:])
                        else:
                            nc.vector.tensor_copy(ob[:], op_[:])
                        nc.sync.dma_start(
                            out_d[c * CH + g * 128: c * CH + (g + 1) * 128,
                                  half * 1024:(half + 1) * 1024], ob[:])

            # ---- pipeline ----------------------------------------------
            dma_startup_head()
            emit_proj(0, "k")
            emit_proj(0, "v")
            issue_xt(1)
            emit_proj(0, "q0")
            emit_proj(0, "q1")
            nc.sync.dma_start(
                wo_sb[:].rearrange("p (h n) -> p h n", h=HQ),
                wo_d.rearrange("(h p) n -> p h n", p=128))
            emit_proj(0, "q2")
            emit_proj(0, "q3")
            for c in range(1, NCH + 1):
                if c < NCH:
                    # interleave chunk c-1 attention with chunk c projections
                    emit_att_head(c - 1, 0)
                    emit_proj(c, "k")
                    emit_att_head(c - 1, 1)
                    emit_proj(c, "v")
                    emit_att_head(c - 1, 2)
                    emit_proj(c, "q0")
                    emit_att_head(c - 1, 3)
                    emit_proj(c, "q1")
                    emit_oproj(c - 1)
                    if c + 1 < NCH:
                        issue_xt(c + 1)
                    emit_proj(c, "q2")
                    emit_proj(c, "q3")
                else:
                    for h in range(HQ):
                        emit_att_head(c - 1, h, alt_pool=(h % 2 == 1))
                    emit_oproj(c - 1)
    nc.finalize()
    return nc


def _get_nc():
    if "nc" not in _cache:
        _cache["nc"] = build_kernel()
    return _cache["nc"]


def kernel(x, wq, wk, wv, wo):
    from concourse.bass_utils import run_bass_kernel_spmd

    x = np.asarray(x, dtype=np.float32)
    shards, c2, s2n = _host_prep(
        np.asarray(wq, np.float32), np.asarray(wk, np.float32),
        np.asarray(wv, np.float32), np.asarray(wo, np.float32))

    ones = np.ones((128, 128), dtype=np.float16)
    x16 = [np.ascontiguousarray(x[b].astype(np.float16)) for b in range(B)]

    in_maps = []
    for c in range(8):
        b = c // 4
        m = dict(shards[c])
        m.update(x16=x16[b], c2=c2, s2n=s2n, ones=ones)
        in_maps.append(m)

    nc = _get_nc()
    res = run_bass_kernel_spmd(
        nc, in_maps, core_ids=list(range(8)),
        trace=bool(int(os.environ.get("KERNEL_TRACE", "0"))),
    )
    _cache["last_result"] = res
    parts = [r["out"].astype(np.float32) for r in res.results]
    out = np.empty((B, S, D), dtype=np.float32)
    for b in range(B):
        out[b] = parts[4 * b] + parts[4 * b + 1] + parts[4 * b + 2] + parts[4 * b + 3]
    return out


# revision 53
# speedup vs baseline: 1.0214x; 1.0214x over previous
"""Trainium2 Bass kernel for a local-attention layer (GQA + RoPE + banded mask).

Full computation (reference semantics, f32):
  q = x@wq, k = x@wk, v = x@wv  (B=2, S=2048, D=2048, Hq=16, Hkv=4, hd=128)
  rope(q), rope(k) interleaved-pair style
  banded causal attention, window=1024, softmax
  out = (probs @ v_rep) @ wo

Sharding: 8 cores = (batch b in {0,1}) x (kv-group g in {0..3}).
Core c handles batch c//4, kv head c%4 and its 4 q heads.  Each core
computes a partial (2048, 2048) f16 output (its heads' contribution
through wo rows); host sums the 4 partials per batch in f32.

Per-core kernel strategy (all matmuls fp16, f32 PSUM accum):
  - X^T pre-transposed on the host (f16) and plain-DMA'd into SBUF --
    no PE transposes, no PSUM staging.
  - Projections computed transposed one head at a time: QT/KT/VT =
    W^T X^T from natural W layout (lhsT=W chunk, rhs=XT chunk).
  - RoPE in "half-split" form on f16 SBUF tiles (host pre-permutes
    wq/wk columns; 1/sqrt(hd) folded into wq): y = x*c2 + swap(x)*s2n
    with duplicated cos and sign-folded sin so every two-input DVE op
    keeps both inputs at base partition 0 (HW verifier rule).
  - V transposed to natural (s, d) tiles by SBUF->SBUF DMA-transpose.
  - Scores transposed ST[j,i] = kt_tile^T @ QT, banded tight: per j-tile
    only the i-subtiles inside the band are computed; exp on ScalarE per
    written window.  Post-exp masks (gpsimd.affine_select, fill 0) only
    on the diagonal and window-edge 128x128 blocks.
  - pv/dn PSUM banks explicitly zeroed (ACT memzero / DVE memset), all
    accumulating matmuls start=False with skip_group_check so ragged
    band windows can accumulate in pipeline order.
  - Softmax denominator via ones[128,128] stationary matmul -> PSUM is
    the broadcast denominator; reciprocal on DVE; normalization fused
    into the PSUM->SBUF copy of the attention output.
  - o-proj accumulates in PSUM, staged to f16 SBUF (ACT/DVE alternating)
    and DMA'd out as f16; host sums the 4 partials per batch.
  - Software pipelining: chunk c's attention/o-proj interleaved with
    chunk c+1's projections; PSUM statically fits 8 banks; startup DMAs
    ordered just-in-time across the SWDGE and two HWDGE queues.
"""

import os
import numpy as np

B, S, D = 2, 2048, 2048
NH, NKV, HD = 16, 4, 128
WINDOW = 1024
ROPE_THETA = 10000.0
HQ = NH // NKV          # q heads per core = 4
QD = HQ * HD            # 512
NK = D // 128           # 16 contraction chunks
CH = 512                # s-chunk size
NCH = S // CH           # 4 chunks
NSUB = CH // 128        # 4 s-subtiles per chunk

_cache = {}


def _host_prep(wq, wk, wv, wo):
    """Per-core f16 weight slices with rope permutation + scale folded in."""
    # de-interleave permutation: dev col j <- ref col (2j if j<64 else 2(j-64)+1)
    perm = np.empty(HD, dtype=np.int64)
    perm[:64] = np.arange(64) * 2
    perm[64:] = np.arange(64) * 2 + 1

    scale = 1.0 / np.sqrt(np.float32(HD))
    wq_p = (wq.reshape(D, NH, HD)[:, :, perm].reshape(D, NH * HD) * scale)
    wk_p = wk.reshape(D, NKV, HD)[:, :, perm].reshape(D, NKV * HD)

    inv_freq = 1.0 / (ROPE_THETA ** (np.arange(0, HD, 2, dtype=np.float32) / HD))
    t = np.arange(S, dtype=np.float32)
    ang = np.outer(t, inv_freq)             # (S, 64)
    cosT = np.cos(ang).T.astype(np.float16)  # (64, S)
    sinT = np.sin(ang).T.astype(np.float16)
    # duplicated cos; sign-folded sin: y = x*c2 + swap(x)*s2n, all ops
    # partition-base aligned (HW: two SBUF inputs must share base partition)
    c2 = np.ascontiguousarray(np.concatenate([cosT, cosT], axis=0))    # (128,S)
    s2n = np.ascontiguousarray(np.concatenate([-sinT, sinT], axis=0))

    shards = []
    for c in range(8):
        g = c % 4
        wkv = np.concatenate(
            [wk_p[:, g * HD:(g + 1) * HD], wv[:, g * HD:(g + 1) * HD]], axis=1)
        sh = dict(
            wkv=np.ascontiguousarray(wkv).astype(np.float16),
            wq01=np.ascontiguousarray(
                wq_p[:, g * QD:g * QD + 2 * HD]).astype(np.float16),
            wq23=np.ascontiguousarray(
                wq_p[:, g * QD + 2 * HD:(g + 1) * QD]).astype(np.float16),
            wo=np.ascontiguousarray(wo[g * QD:(g + 1) * QD, :]).astype(np.float16),
        )
        shards.append(sh)
    return shards, c2, s2n


def _jt_win(c, jt):
    """Allowed i-subtile window [lo, hi] (global isub idx) for j-tile jt in chunk c."""
    lo = max(4 * c, jt)
    hi = min(4 * c + 3, jt + WINDOW // 128)
    return lo, hi


def build_kernel():
    import concourse.bass as bass
    import concourse.mybir as mybir
    import concourse.tile as tile
    from concourse import bacc

    f16 = mybir.dt.float16
    f32 = mybir.dt.float32
    EXP = mybir.ActivationFunctionType.Exp
    GE = mybir.AluOpType.is_ge
    NW = WINDOW // 128      # 8

    nc = bacc.Bacc("TRN2", target_bir_lowering=False, debug=False, num_devices=8)

    xt16_d = nc.dram_tensor("xt16", [D, S], f16, kind="ExternalInput").ap()
    wkv_d = nc.dram_tensor("wkv", [D, 2 * HD], f16, kind="ExternalInput").ap()
    wq01_d = nc.dram_tensor("wq01", [D, 2 * HD], f16, kind="ExternalInput").ap()
    wq23_d = nc.dram_tensor("wq23", [D, 2 * HD], f16, kind="ExternalInput").ap()
    wo_d = nc.dram_tensor("wo", [QD, D], f16, kind="ExternalInput").ap()
    c_d = nc.dram_tensor("c2", [128, S], f16, kind="ExternalInput").ap()
    s_d = nc.dram_tensor("s2n", [128, S], f16, kind="ExternalInput").ap()
    one_d = nc.dram_tensor("ones", [128, 128], f16, kind="ExternalInput").ap()
    out_d = nc.dram_tensor("out", [S, D], f16, kind="ExternalOutput").ap()

    with tile.TileContext(nc) as tc:
        with (
            tc.tile_pool(name="persist", bufs=1) as pp,
            tc.tile_pool(name="xtpool", bufs=2) as xtp,
            tc.tile_pool(name="qtpool", bufs=8) as qtp,
            tc.tile_pool(name="ropet", bufs=2) as rtp,
            tc.tile_pool(name="ropeab", bufs=4) as rab,
            tc.tile_pool(name="vtpool", bufs=2) as vtp,
            tc.tile_pool(name="ptpool", bufs=8) as ptp,
            tc.tile_pool(name="atpool", bufs=6) as atp,
            tc.tile_pool(name="rcpool", bufs=2) as rcp_p,
            tc.tile_pool(name="obpool", bufs=3) as obp,
            tc.tile_pool(name="psProj", bufs=2, space="PSUM") as pjp,
            tc.tile_pool(name="psBig", bufs=2, space="PSUM") as stp,
            tc.tile_pool(name="psPV", bufs=1, space="PSUM") as pvp,
            tc.tile_pool(name="psDN", bufs=1, space="PSUM") as dnp,
        ):
            # ---- persistent SBUF tensors -------------------------------
            wkv_sb = pp.tile([128, NK * 2 * HD], f16, tag="wkv")  # [k][wk|wv]
            wq_sb = pp.tile([128, NK * QD], f16, tag="wq")        # [k][:, qd]
            wo_sb = pp.tile([128, HQ * D], f16, tag="wo")       # [h][:, e]
            c_sb = pp.tile([128, S], f16, tag="c")
            s_sb = pp.tile([128, S], f16, tag="s")
            one_sb = pp.tile([128, 128], f16, tag="one")
            kt_sb = pp.tile([128, S], f16, tag="kt")            # rope'd K^T
            v_sb = pp.tile([128, NK * 128], f16, tag="v")       # [jt][s, d]

            xts = [None] * NCH

            def issue_xt(c, nsub=2):
                """Load host-pre-transposed X^T k-tiles for chunk c,
                split so early k-tiles land sooner."""
                xt = xtp.tile([128, NK * CH], f16, tag="xt", name=f"xt{c}")
                kn = NK // nsub
                for q in range(nsub):
                    nc.sync.dma_start(
                        xt[:, kn * q * CH:kn * (q + 1) * CH]
                        .rearrange("p (k s) -> p k s", k=kn),
                        xt16_d[kn * q * 128:kn * (q + 1) * 128,
                               c * CH:(c + 1) * CH]
                        .rearrange("(k p) s -> p k s", p=128))
                xts[c] = xt

            # startup DMA order matters: the DMA transfers serialize on a
            # shared device in arrival order, so issue just-in-time for
            # the chunk-0 pipeline (interleaved with the proj passes below).
            def dma_startup_head():
                # wkv via SWDGE (parallel with HWDGE), split in two so
                # the first k-tiles land sooner; x^T(0) quarters alternate
                # across the two HWDGE queues
                nc.gpsimd.dma_start(
                    wkv_sb[:, 0:NK * HD].rearrange("p (k n) -> p k n", k=NK // 2),
                    wkv_d[0:D // 2, :].rearrange("(k p) n -> p k n", p=128))
                nc.gpsimd.dma_start(
                    wkv_sb[:, NK * HD:].rearrange("p (k n) -> p k n", k=NK // 2),
                    wkv_d[D // 2:, :].rearrange("(k p) n -> p k n", p=128))
                xt = xtp.tile([128, NK * CH], f16, tag="xt", name="xt0")
                for q in range(4):
                    eng = nc.sync if q % 2 == 0 else nc.scalar
                    eng.dma_start(
                        xt[:, 4 * q * CH:4 * (q + 1) * CH]
                        .rearrange("p (k s) -> p k s", k=4),
                        xt16_d[4 * q * 128:4 * (q + 1) * 128, 0:CH]
                        .rearrange("(k p) s -> p k s", p=128))
                xts[0] = xt
                # trig + ones via SWDGE: lands early, in parallel with the
                # HWDGE stream (the rope ops head the DVE queue and would
                # otherwise stall all DVE retirement on these)
                nc.gpsimd.dma_start(c_sb[:], c_d)
                nc.gpsimd.dma_start(s_sb[:], s_d)
                nc.gpsimd.dma_start(one_sb[:], one_d)
                nc.scalar.dma_start(
                    wq_sb[:, 0:NK * 2 * HD]
                    .rearrange("p (k n) -> p k n", k=NK),
                    wq01_d.rearrange("(k p) n -> p k n", p=128))
                nc.scalar.dma_start(
                    wq_sb[:, NK * 2 * HD:]
                    .rearrange("p (k n) -> p k n", k=NK),
                    wq23_d.rearrange("(k p) n -> p k n", p=128))

            def rope(dst, src, c):
                """src (128, CH) f16 sbuf -> dst (128, CH) f16, half-split rope:
                dst = src*c2 + swap(src)*s2n.  Two-input DVE ops keep both
                inputs at base partition 0 (HW verifier requirement)."""
                cs = c_sb[:, c * CH:(c + 1) * CH]
                sn = s_sb[:, c * CH:(c + 1) * CH]
                sw = rab.tile([128, CH], f16, tag="ra")
                nc.vector.tensor_copy(sw[0:64, :], src[64:128, :])
                nc.vector.tensor_copy(sw[64:128, :], src[0:64, :])
                a = rab.tile([128, CH], f16, tag="ra")
                nc.vector.tensor_mul(a[:], src[:], cs)
                nc.vector.tensor_mul(sw[:], sw[:], sn)
                nc.vector.tensor_add(dst[:], a[:], sw[:])

            def proj_pass_off(c, w_sb, off, stride):
                """One 128-wide projection: returns f16 copy in SBUF."""
                ps = pjp.tile([128, CH], f32, tag="pj", name=f"pj{c}_{off}")
                xt = xts[c]
                for k in range(NK):
                    nc.tensor.matmul(
                        ps[:],
                        w_sb[:, k * stride + off: k * stride + off + 128],
                        xt[:, k * CH:(k + 1) * CH],
                        start=(k == 0), stop=(k == NK - 1),
                    )
                xf = rtp.tile([128, CH], f16, tag="xf", name=f"xf{c}_{off}")
                nc.vector.tensor_copy(xf[:], ps[:])
                return xf

            proj_pass = proj_pass_off

            def emit_proj(c, which):
                """which in {'k','v','q0'..'q3'}"""
                if which == "k":
                    xf = proj_pass_off(c, wkv_sb, 0, 2 * HD)
                    rope(kt_sb[:, c * CH:(c + 1) * CH], xf, c)
                elif which == "v":
                    ps = pjp.tile([128, CH], f32, tag="pj", name=f"pjv{c}")
                    xt = xts[c]
                    for k in range(NK):
                        nc.tensor.matmul(
                            ps[:], wkv_sb[:, k * 2 * HD + HD: (k + 1) * 2 * HD],
                            xt[:, k * CH:(k + 1) * CH],
                            start=(k == 0), stop=(k == NK - 1),
                        )
                    vt = vtp.tile([128, CH], f16, tag="vt", name=f"vt{c}")
                    nc.scalar.copy(vt[:], ps[:])
                    # natural (s, d) tiles via SBUF->SBUF DMA transpose
                    nc.sync.dma_start_transpose(
                        v_sb[:, c * NSUB * 128:(c + 1) * NSUB * 128]
                        .rearrange("p (g d) -> p g d", g=NSUB),
                        vt[:])
                else:
                    h = int(which[1])
                    off = (h // 2) * NK * 2 * HD
                    xf = proj_pass_off(c, wq_sb, off + (h % 2) * HD, 2 * HD)
                    qt = qtp.tile([128, CH], f16, tag="qt", name=f"qt{c}_{h}")
                    rope(qt, xf, c)
                    qts[(c, h)] = qt

            qts = {}
            ats = {}

            def emit_att_head(c, h, alt_pool=False):
                jt0 = max(0, 4 * c - NW)
                jts = list(range(jt0, 4 * c + 4))
                # tail chunk: odd heads borrow the (idle) projection PSUM
                # pool so head boundaries double-buffer pv/dn
                pool_v = pjp if alt_pool else pvp
                pool_d = pjp if alt_pool else dnp
                pv_ps = pool_v.tile([128, CH], f32, tag="pj" if alt_pool else "pv",
                                    name=f"pv{c}_{h}")
                dn_ps = pool_d.tile([128, CH], f32, tag="pj" if alt_pool else "dn",
                                    name=f"dn{c}_{h}")
                # explicit zero + start=False accumulation everywhere: lets
                # ragged-window matmuls accumulate in natural (pipeline) order
                nc.scalar.memzero(pv_ps[:])
                nc.vector.memset(dn_ps[:], 0.0)
                pts = []
                for pi in range(0, len(jts), 2):
                    pair = jts[pi:pi + 2]
                    st = stp.tile([128, 2 * CH], f32, tag="big", name=f"st{c}_{h}_{pi}")
                    for q_, jt in enumerate(pair):
                        lo, hi = _jt_win(c, jt)
                        a0 = (lo - 4 * c) * 128
                        a1 = (hi - 4 * c + 1) * 128
                        nc.tensor.matmul(
                            st[:, q_ * CH + a0: q_ * CH + a1],
                            kt_sb[:, jt * 128:(jt + 1) * 128],
                            qts[(c, h)][:, a0:a1],
                            start=True, stop=True,
                        )
                    pt = ptp.tile([128, 2 * CH], f16, tag="pt", name=f"pt{c}_{h}_{pi}")
                    for q_, jt in enumerate(pair):
                        lo, hi = _jt_win(c, jt)
                        a0 = q_ * CH + (lo - 4 * c) * 128
                        a1 = q_ * CH + (hi - 4 * c + 1) * 128
                        nc.scalar.activation(pt[:, a0:a1], st[:, a0:a1], EXP)
                    for q_, jt in enumerate(pair):
                        if 4 * c <= jt:          # diagonal block: keep j <= i
                            o = q_ * CH + (jt - 4 * c) * 128
                            nc.gpsimd.affine_select(
                                out=pt[:, o:o + 128], in_=pt[:, o:o + 128],
                                pattern=[[1, 128]], compare_op=GE, fill=0.0,
                                base=0, channel_multiplier=-1,
                            )
                        if jt + NW <= 4 * c + 3:  # window edge: keep j > i-W-1
                            o = q_ * CH + (jt + NW - 4 * c) * 128
                            nc.gpsimd.affine_select(
                                out=pt[:, o:o + 128], in_=pt[:, o:o + 128],
                                pattern=[[-1, 128]], compare_op=GE, fill=0.0,
                                base=0, channel_multiplier=1,
                            )
                    pts.append((pair, pt))

                items = []
                for pi, (pair, pt) in enumerate(pts):
                    for q_, jt in enumerate(pair):
                        lo, hi = _jt_win(c, jt)
                        items.append((jt, (lo - 4 * c) * 128,
                                      (hi - 4 * c + 1) * 128, q_, pt))
                for n_, (jt, a0, a1, q_, pt) in enumerate(items):
                    nc.tensor.matmul(
                        pv_ps[:, a0:a1],
                        v_sb[:, jt * 128:(jt + 1) * 128],
                        pt[:, q_ * CH + a0: q_ * CH + a1],
                        start=False, stop=(n_ == len(items) - 1),
                        skip_group_check=True,
                    )
                for n_, (jt, a0, a1, q_, pt) in enumerate(items):
                    nc.tensor.matmul(
                        dn_ps[:, a0:a1],
                        one_sb[:],
                        pt[:, q_ * CH + a0: q_ * CH + a1],
                        start=False, stop=(n_ == len(items) - 1),
                        skip_group_check=True,
                    )
                rcp = rcp_p.tile([128, CH], f32, tag="rcp", name=f"rcp{c}_{h}")
                nc.vector.reciprocal(rcp[:], dn_ps[:])
                at = atp.tile([128, CH], f16, tag="at", name=f"at{c}_{h}")
                nc.vector.tensor_mul(at[:], pv_ps[:], rcp[:])
                ats[(c, h)] = at

            def emit_oproj(c):
                for g in range(NSUB):
                    for half in range(2):
                        op_ = stp.tile([128, 2 * CH], f32, tag="big",
                                       name=f"op{c}_{g}_{half}")
                        for e2 in range(2):
                            ecol = half * 1024 + e2 * CH
                            for h in range(HQ):
                                nc.tensor.matmul(
                                    op_[:, e2 * CH:(e2 + 1) * CH],
                                    ats[(c, h)][:, g * 128:(g + 1) * 128],
                                    wo_sb[:, h * D + ecol: h * D + ecol + CH],
                                    start=(h == 0), stop=(h == HQ - 1),
                                )
                        ob = obp.tile([128, 2 * CH], f16, tag="ob",
                                      name=f"ob{c}_{g}_{half}")
                        if half == 0:
                            nc.scalar.copy(ob[:], op_[:])
                        else:
                            nc.vector.tensor_copy(ob[:], op_[:])
                        nc.sync.dma_start(
                            out_d[c * CH + g * 128: c * CH + (g + 1) * 128,
                                  half * 1024:(half + 1) * 1024], ob[:])

            # ---- pipeline ----------------------------------------------
            dma_startup_head()
            emit_proj(0, "k")
            emit_proj(0, "v")
            issue_xt(1)
            emit_proj(0, "q0")
            emit_proj(0, "q1")
            nc.sync.dma_start(
                wo_sb[:].rearrange("p (h n) -> p h n", h=HQ),
                wo_d.rearrange("(h p) n -> p h n", p=128))
            emit_proj(0, "q2")
            emit_proj(0, "q3")
            for c in range(1, NCH + 1):
                if c < NCH:
                    # interleave chunk c-1 attention with chunk c projections
                    emit_att_head(c - 1, 0)
                    emit_proj(c, "k")
                    emit_att_head(c - 1, 1)
                    emit_proj(c, "v")
                    emit_att_head(c - 1, 2)
                    emit_proj(c, "q0")
                    emit_att_head(c - 1, 3)
                    emit_proj(c, "q1")
                    emit_oproj(c - 1)
                    if c + 1 < NCH:
                        issue_xt(c + 1)
                    emit_proj(c, "q2")
                    emit_proj(c, "q3")
                else:
                    for h in range(HQ):
                        emit_att_head(c - 1, h, alt_pool=(h % 2 == 1))
                    emit_oproj(c - 1)
    nc.finalize()
    return nc


def _get_nc():
    if "nc" not in _cache:
        _cache["nc"] = build_kernel()
    return _cache["nc"]


def kernel(x, wq, wk, wv, wo):
    from concourse.bass_utils import run_bass_kernel_spmd

    x = np.asarray(x, dtype=np.float32)
    shards, c2, s2n = _host_prep(
        np.asarray(wq, np.float32), np.asarray(wk, np.float32),
        np.asarray(wv, np.float32), np.asarray(wo, np.float32))

    ones = np.ones((128, 128), dtype=np.float16)
    xt16 = [np.ascontiguousarray(x[b].T.astype(np.float16)) for b in range(B)]

    in_maps = []
    for c in range(8):
        b = c // 4
        m = dict(shards[c])
        m.update(xt16=xt16[b], c2=c2, s2n=s2n, ones=ones)
        in_maps.append(m)

    nc = _get_nc()
    res = run_bass_kernel_spmd(
        nc, in_maps, core_ids=list(range(8)),
        trace=bool(int(os.environ.get("KERNEL_TRACE", "0"))),
    )
    _cache["last_result"] = res
    parts = [r["out"].astype(np.float32) for r in res.results]
    out = np.empty((B, S, D), dtype=np.float32)
    for b in range(B):
        out[b] = parts[4 * b] + parts[4 * b + 1] + parts[4 * b + 2] + parts[4 * b + 3]
    return out


# revision 58
# speedup vs baseline: 1.0379x; 1.0162x over previous
"""Trainium2 Bass kernel for a local-attention layer (GQA + RoPE + banded mask).

Full computation (reference semantics, f32):
  q = x@wq, k = x@wk, v = x@wv  (B=2, S=2048, D=2048, Hq=16, Hkv=4, hd=128)
  rope(q), rope(k) interleaved-pair style
  banded causal attention, window=1024, softmax
  out = (probs @ v_rep) @ wo

Sharding: 8 cores = (batch b in {0,1}) x (kv-group g in {0..3}).
Core c handles batch c//4, kv head c%4 and its 4 q heads.  Each core
computes a partial (2048, 2048) f16 output (its heads' contribution
through wo rows); host sums the 4 partials per batch in f32.

Per-core kernel strategy (all matmuls fp16, f32 PSUM accum):
  - X^T pre-transposed on the host (f16) and plain-DMA'd into SBUF --
    no PE transposes, no PSUM staging.
  - Projections computed transposed one head at a time: QT/KT/VT =
    W^T X^T from natural W layout (lhsT=W chunk, rhs=XT chunk).
  - RoPE in "half-split" form on f16 SBUF tiles (host pre-permutes
    wq/wk columns; 1/sqrt(hd) folded into wq): y = x*c2 + swap(x)*s2n
    with duplicated cos and sign-folded sin so every two-input DVE op
    keeps both inputs at base partition 0 (HW verifier rule).
  - V transposed to natural (s, d) tiles by SBUF->SBUF DMA-transpose.
  - Scores transposed ST[j,i] = kt_tile^T @ QT, banded tight: per j-tile
    only the i-subtiles inside the band are computed; exp on ScalarE per
    written window.  Post-exp masks (gpsimd.affine_select, fill 0) only
    on the diagonal and window-edge 128x128 blocks.
  - pv/dn PSUM banks explicitly zeroed (ACT memzero / DVE memset), all
    accumulating matmuls start=False with skip_group_check so ragged
    band windows can accumulate in pipeline order.
  - Softmax denominator via ones[128,128] stationary matmul -> PSUM is
    the broadcast denominator; reciprocal on DVE; normalization fused
    into the PSUM->SBUF copy of the attention output.
  - o-proj accumulates in PSUM, staged to f16 SBUF (ACT/DVE alternating)
    and DMA'd out as f16; host sums the 4 partials per batch.
  - Software pipelining: chunk c's attention/o-proj interleaved with
    chunk c+1's projections; PSUM statically fits 8 banks; startup DMAs
    ordered just-in-time across the SWDGE and two HWDGE queues.
"""

import os
import numpy as np

B, S, D = 2, 2048, 2048
NH, NKV, HD = 16, 4, 128
WINDOW = 1024
ROPE_THETA = 10000.0
HQ = NH // NKV          # q heads per core = 4
QD = HQ * HD            # 512
NK = D // 128           # 16 contraction chunks
CH = 512                # s-chunk size
NCH = S // CH           # 4 chunks
NSUB = CH // 128        # 4 s-subtiles per chunk

_cache = {}


def _host_prep(wq, wk, wv, wo):
    """Per-core f16 weight slices with rope permutation + scale folded in."""
    # de-interleave permutation: dev col j <- ref col (2j if j<64 else 2(j-64)+1)
    perm = np.empty(HD, dtype=np.int64)
    perm[:64] = np.arange(64) * 2
    perm[64:] = np.arange(64) * 2 + 1

    scale = 1.0 / np.sqrt(np.float32(HD))
    wq_p = (wq.reshape(D, NH, HD)[:, :, perm].reshape(D, NH * HD) * scale)
    wk_p = wk.reshape(D, NKV, HD)[:, :, perm].reshape(D, NKV * HD)

    inv_freq = 1.0 / (ROPE_THETA ** (np.arange(0, HD, 2, dtype=np.float32) / HD))
    t = np.arange(S, dtype=np.float32)
    ang = np.outer(t, inv_freq)             # (S, 64)
    cosT = np.cos(ang).T.astype(np.float16)  # (64, S)
    sinT = np.sin(ang).T.astype(np.float16)
    # duplicated cos; sign-folded sin: y = x*c2 + swap(x)*s2n, all ops
    # partition-base aligned (HW: two SBUF inputs must share base partition)
    c2 = np.ascontiguousarray(np.concatenate([cosT, cosT], axis=0))    # (128,S)
    s2n = np.ascontiguousarray(np.concatenate([-sinT, sinT], axis=0))

    shards = []
    for c in range(8):
        g = c % 4
        wkv = np.concatenate(
            [wk_p[:, g * HD:(g + 1) * HD], wv[:, g * HD:(g + 1) * HD]], axis=1)
        sh = dict(
            wkv=np.ascontiguousarray(wkv).astype(np.float16),
            wq01=np.ascontiguousarray(
                wq_p[:, g * QD:g * QD + 2 * HD]).astype(np.float16),
            wq23=np.ascontiguousarray(
                wq_p[:, g * QD + 2 * HD:(g + 1) * QD]).astype(np.float16),
            wo=np.ascontiguousarray(wo[g * QD:(g + 1) * QD, :]).astype(np.float16),
        )
        shards.append(sh)
    return shards, c2, s2n


def _jt_win(c, jt):
    """Allowed i-subtile window [lo, hi] (global isub idx) for j-tile jt in chunk c."""
    lo = max(4 * c, jt)
    hi = min(4 * c + 3, jt + WINDOW // 128)
    return lo, hi


def build_kernel():
    import concourse.bass as bass
    import concourse.mybir as mybir
    import concourse.tile as tile
    from concourse import bacc

    f16 = mybir.dt.float16
    f32 = mybir.dt.float32
    EXP = mybir.ActivationFunctionType.Exp
    GE = mybir.AluOpType.is_ge
    NW = WINDOW // 128      # 8

    nc = bacc.Bacc("TRN2", target_bir_lowering=False, debug=False, num_devices=8)

    xt16_d = nc.dram_tensor("xt16", [D, S], f16, kind="ExternalInput").ap()
    wkv_d = nc.dram_tensor("wkv", [D, 2 * HD], f16, kind="ExternalInput").ap()
    wq01_d = nc.dram_tensor("wq01", [D, 2 * HD], f16, kind="ExternalInput").ap()
    wq23_d = nc.dram_tensor("wq23", [D, 2 * HD], f16, kind="ExternalInput").ap()
    wo_d = nc.dram_tensor("wo", [QD, D], f16, kind="ExternalInput").ap()
    c_d = nc.dram_tensor("c2", [128, S], f16, kind="ExternalInput").ap()
    s_d = nc.dram_tensor("s2n", [128, S], f16, kind="ExternalInput").ap()
    one_d = nc.dram_tensor("ones", [128, 128], f16, kind="ExternalInput").ap()
    out_d = nc.dram_tensor("out", [S, D], f16, kind="ExternalOutput").ap()

    with tile.TileContext(nc) as tc:
        with (
            tc.tile_pool(name="persist", bufs=1) as pp,
            tc.tile_pool(name="xtpool", bufs=2) as xtp,
            tc.tile_pool(name="qtpool", bufs=8) as qtp,
            tc.tile_pool(name="ropet", bufs=2) as rtp,
            tc.tile_pool(name="ropeab", bufs=4) as rab,
            tc.tile_pool(name="vtpool", bufs=2) as vtp,
            tc.tile_pool(name="ptpool", bufs=8) as ptp,
            tc.tile_pool(name="atpool", bufs=6) as atp,
            tc.tile_pool(name="rcpool", bufs=2) as rcp_p,
            tc.tile_pool(name="obpool", bufs=3) as obp,
            tc.tile_pool(name="psProj", bufs=2, space="PSUM") as pjp,
            tc.tile_pool(name="psBig", bufs=2, space="PSUM") as stp,
            tc.tile_pool(name="psPV", bufs=1, space="PSUM") as pvp,
            tc.tile_pool(name="psDN", bufs=1, space="PSUM") as dnp,
        ):
            # ---- persistent SBUF tensors -------------------------------
            wkv_sb = pp.tile([128, NK * 2 * HD], f16, tag="wkv")  # [k][wk|wv]
            wq_sb = pp.tile([128, NK * QD], f16, tag="wq")        # [k][:, qd]
            wo_sb = pp.tile([128, HQ * D], f16, tag="wo")       # [h][:, e]
            c_sb = pp.tile([128, S], f16, tag="c")
            s_sb = pp.tile([128, S], f16, tag="s")
            one_sb = pp.tile([128, 128], f16, tag="one")
            kt_sb = pp.tile([128, S], f16, tag="kt")            # rope'd K^T
            v_sb = pp.tile([128, NK * 128], f16, tag="v")       # [jt][s, d]

            xts = [None] * NCH

            def issue_xt(c, nsub=2):
                """Load host-pre-transposed X^T k-tiles for chunk c,
                split so early k-tiles land sooner."""
                xt = xtp.tile([128, NK * CH], f16, tag="xt", name=f"xt{c}")
                kn = NK // nsub
                for q in range(nsub):
                    nc.sync.dma_start(
                        xt[:, kn * q * CH:kn * (q + 1) * CH]
                        .rearrange("p (k s) -> p k s", k=kn),
                        xt16_d[kn * q * 128:kn * (q + 1) * 128,
                               c * CH:(c + 1) * CH]
                        .rearrange("(k p) s -> p k s", p=128))
                xts[c] = xt

            # startup DMA order matters: the DMA transfers serialize on a
            # shared device in arrival order, so issue just-in-time for
            # the chunk-0 pipeline (interleaved with the proj passes below).
            def dma_startup_head():
                # wkv via SWDGE (parallel with HWDGE), split in two so
                # the first k-tiles land sooner; x^T(0) quarters alternate
                # across the two HWDGE queues
                for q in range(4):
                    w0 = q * (NK // 4) * 2 * HD
                    r0 = q * (D // 4)
                    nc.gpsimd.dma_start(
                        wkv_sb[:, w0:w0 + (NK // 4) * 2 * HD]
                        .rearrange("p (k n) -> p k n", k=NK // 4),
                        wkv_d[r0:r0 + D // 4, :]
                        .rearrange("(k p) n -> p k n", p=128))
                xt = xtp.tile([128, NK * CH], f16, tag="xt", name="xt0")
                for q in range(4):
                    eng = nc.sync if q % 2 == 0 else nc.scalar
                    eng.dma_start(
                        xt[:, 4 * q * CH:4 * (q + 1) * CH]
                        .rearrange("p (k s) -> p k s", k=4),
                        xt16_d[4 * q * 128:4 * (q + 1) * 128, 0:CH]
                        .rearrange("(k p) s -> p k s", p=128))
                xts[0] = xt
                # trig + ones via SWDGE: lands early, in parallel with the
                # HWDGE stream (the rope ops head the DVE queue and would
                # otherwise stall all DVE retirement on these)
                nc.gpsimd.dma_start(c_sb[:], c_d)
                nc.gpsimd.dma_start(s_sb[:], s_d)
                nc.gpsimd.dma_start(one_sb[:], one_d)
                nc.scalar.dma_start(
                    wq_sb[:, 0:NK * 2 * HD]
                    .rearrange("p (k n) -> p k n", k=NK),
                    wq01_d.rearrange("(k p) n -> p k n", p=128))
                nc.scalar.dma_start(
                    wq_sb[:, NK * 2 * HD:]
                    .rearrange("p (k n) -> p k n", k=NK),
                    wq23_d.rearrange("(k p) n -> p k n", p=128))

            def rope(dst, src, c):
                """src (128, CH) f16 sbuf -> dst (128, CH) f16, half-split rope:
                dst = src*c2 + swap(src)*s2n.  Two-input DVE ops keep both
                inputs at base partition 0 (HW verifier requirement)."""
                cs = c_sb[:, c * CH:(c + 1) * CH]
                sn = s_sb[:, c * CH:(c + 1) * CH]
                sw = rab.tile([128, CH], f16, tag="ra")
                nc.vector.tensor_copy(sw[0:64, :], src[64:128, :])
                nc.vector.tensor_copy(sw[64:128, :], src[0:64, :])
                a = rab.tile([128, CH], f16, tag="ra")
                nc.vector.tensor_mul(a[:], src[:], cs)
                nc.vector.tensor_mul(sw[:], sw[:], sn)
                nc.vector.tensor_add(dst[:], a[:], sw[:])

            def proj_pass_off(c, w_sb, off, stride):
                """One 128-wide projection: returns f16 copy in SBUF."""
                ps = pjp.tile([128, CH], f32, tag="pj", name=f"pj{c}_{off}")
                xt = xts[c]
                for k in range(NK):
                    nc.tensor.matmul(
                        ps[:],
                        w_sb[:, k * stride + off: k * stride + off + 128],
                        xt[:, k * CH:(k + 1) * CH],
                        start=(k == 0), stop=(k == NK - 1),
                    )
                xf = rtp.tile([128, CH], f16, tag="xf", name=f"xf{c}_{off}")
                nc.vector.tensor_copy(xf[:], ps[:])
                return xf

            proj_pass = proj_pass_off

            def emit_proj(c, which):
                """which in {'k','v','q0'..'q3'}"""
                if which == "k":
                    xf = proj_pass_off(c, wkv_sb, 0, 2 * HD)
                    rope(kt_sb[:, c * CH:(c + 1) * CH], xf, c)
                elif which == "v":
                    ps = pjp.tile([128, CH], f32, tag="pj", name=f"pjv{c}")
                    xt = xts[c]
                    for k in range(NK):
                        nc.tensor.matmul(
                            ps[:], wkv_sb[:, k * 2 * HD + HD: (k + 1) * 2 * HD],
                            xt[:, k * CH:(k + 1) * CH],
                            start=(k == 0), stop=(k == NK - 1),
                        )
                    vt = vtp.tile([128, CH], f16, tag="vt", name=f"vt{c}")
                    nc.scalar.copy(vt[:], ps[:])
                    # natural (s, d) tiles via SBUF->SBUF DMA transpose
                    nc.sync.dma_start_transpose(
                        v_sb[:, c * NSUB * 128:(c + 1) * NSUB * 128]
                        .rearrange("p (g d) -> p g d", g=NSUB),
                        vt[:])
                else:
                    h = int(which[1])
                    off = (h // 2) * NK * 2 * HD
                    xf = proj_pass_off(c, wq_sb, off + (h % 2) * HD, 2 * HD)
                    qt = qtp.tile([128, CH], f16, tag="qt", name=f"qt{c}_{h}")
                    rope(qt, xf, c)
                    qts[(c, h)] = qt

            qts = {}
            ats = {}

            def emit_att_head(c, h, alt_pool=False):
                jt0 = max(0, 4 * c - NW)
                jts = list(range(jt0, 4 * c + 4))
                # tail chunk: odd heads borrow the (idle) projection PSUM
                # pool so head boundaries double-buffer pv/dn
                pool_v = pjp if alt_pool else pvp
                pool_d = pjp if alt_pool else dnp
                pv_ps = pool_v.tile([128, CH], f32, tag="pj" if alt_pool else "pv",
                                    name=f"pv{c}_{h}")
                dn_ps = pool_d.tile([128, CH], f32, tag="pj" if alt_pool else "dn",
                                    name=f"dn{c}_{h}")
                # explicit zero + start=False accumulation everywhere: lets
                # ragged-window matmuls accumulate in natural (pipeline) order
                nc.vector.memset(pv_ps[:], 0.0)
                nc.vector.memset(dn_ps[:], 0.0)
                pts = []
                for pi in range(0, len(jts), 2):
                    pair = jts[pi:pi + 2]
                    st = stp.tile([128, 2 * CH], f32, tag="big", name=f"st{c}_{h}_{pi}")
                    for q_, jt in enumerate(pair):
                        lo, hi = _jt_win(c, jt)
                        a0 = (lo - 4 * c) * 128
                        a1 = (hi - 4 * c + 1) * 128
                        nc.tensor.matmul(
                            st[:, q_ * CH + a0: q_ * CH + a1],
                            kt_sb[:, jt * 128:(jt + 1) * 128],
                            qts[(c, h)][:, a0:a1],
                            start=True, stop=True,
                        )
                    pt = ptp.tile([128, 2 * CH], f16, tag="pt", name=f"pt{c}_{h}_{pi}")
                    for q_, jt in enumerate(pair):
                        lo, hi = _jt_win(c, jt)
                        a0 = q_ * CH + (lo - 4 * c) * 128
                        a1 = q_ * CH + (hi - 4 * c + 1) * 128
                        nc.scalar.activation(pt[:, a0:a1], st[:, a0:a1], EXP)
                    for q_, jt in enumerate(pair):
                        if 4 * c <= jt:          # diagonal block: keep j <= i
                            o = q_ * CH + (jt - 4 * c) * 128
                            nc.gpsimd.affine_select(
                                out=pt[:, o:o + 128], in_=pt[:, o:o + 128],
                                pattern=[[1, 128]], compare_op=GE, fill=0.0,
                                base=0, channel_multiplier=-1,
                            )
                        if jt + NW <= 4 * c + 3:  # window edge: keep j > i-W-1
                            o = q_ * CH + (jt + NW - 4 * c) * 128
                            nc.gpsimd.affine_select(
                                out=pt[:, o:o + 128], in_=pt[:, o:o + 128],
                                pattern=[[-1, 128]], compare_op=GE, fill=0.0,
                                base=0, channel_multiplier=1,
                            )
                    pts.append((pair, pt))

                items = []
                for pi, (pair, pt) in enumerate(pts):
                    for q_, jt in enumerate(pair):
                        lo, hi = _jt_win(c, jt)
                        items.append((jt, (lo - 4 * c) * 128,
                                      (hi - 4 * c + 1) * 128, q_, pt))
                for n_, (jt, a0, a1, q_, pt) in enumerate(items):
                    nc.tensor.matmul(
                        pv_ps[:, a0:a1],
                        v_sb[:, jt * 128:(jt + 1) * 128],
                        pt[:, q_ * CH + a0: q_ * CH + a1],
                        start=False, stop=(n_ == len(items) - 1),
                        skip_group_check=True,
                    )
                for n_, (jt, a0, a1, q_, pt) in enumerate(items):
                    nc.tensor.matmul(
                        dn_ps[:, a0:a1],
                        one_sb[:],
                        pt[:, q_ * CH + a0: q_ * CH + a1],
                        start=False, stop=(n_ == len(items) - 1),
                        skip_group_check=True,
                    )
                rcp = rcp_p.tile([128, CH], f32, tag="rcp", name=f"rcp{c}_{h}")
                nc.vector.reciprocal(rcp[:], dn_ps[:])
                at = atp.tile([128, CH], f16, tag="at", name=f"at{c}_{h}")
                nc.vector.tensor_mul(at[:], pv_ps[:], rcp[:])
                ats[(c, h)] = at

            def emit_oproj(c):
                for g in range(NSUB):
                    for half in range(2):
                        op_ = stp.tile([128, 2 * CH], f32, tag="big",
                                       name=f"op{c}_{g}_{half}")
                        for e2 in range(2):
                            ecol = half * 1024 + e2 * CH
                            for h in range(HQ):
                                nc.tensor.matmul(
                                    op_[:, e2 * CH:(e2 + 1) * CH],
                                    ats[(c, h)][:, g * 128:(g + 1) * 128],
                                    wo_sb[:, h * D + ecol: h * D + ecol + CH],
                                    start=(h == 0), stop=(h == HQ - 1),
                                )
                        ob = obp.tile([128, 2 * CH], f16, tag="ob",
                                      name=f"ob{c}_{g}_{half}")
                        if half == 0:
                            nc.scalar.copy(ob[:], op_[:])
                        else:
                            nc.vector.tensor_copy(ob[:], op_[:])
                        nc.sync.dma_start(
                            out_d[c * CH + g * 128: c * CH + (g + 1) * 128,
                                  half * 1024:(half + 1) * 1024], ob[:])

            # ---- pipeline ----------------------------------------------
            dma_startup_head()
            emit_proj(0, "k")
            emit_proj(0, "v")
            issue_xt(1)
            emit_proj(0, "q0")
            emit_proj(0, "q1")
            nc.sync.dma_start(
                wo_sb[:].rearrange("p (h n) -> p h n", h=HQ),
                wo_d.rearrange("(h p) n -> p h n", p=128))
            emit_proj(0, "q2")
            emit_proj(0, "q3")
            for c in range(1, NCH + 1):
                if c < NCH:
                    # interleave chunk c-1 attention with chunk c projections
                    emit_att_head(c - 1, 0)
                    emit_proj(c, "k")
                    emit_att_head(c - 1, 1)
                    emit_proj(c, "v")
                    emit_att_head(c - 1, 2)
                    emit_proj(c, "q0")
                    emit_att_head(c - 1, 3)
                    emit_proj(c, "q1")
                    emit_oproj(c - 1)
                    if c + 1 < NCH:
                        issue_xt(c + 1)
                    emit_proj(c, "q2")
                    emit_proj(c, "q3")
                else:
                    for h in range(HQ):
                        emit_att_head(c - 1, h, alt_pool=(h % 2 == 1))
                    emit_oproj(c - 1)
    nc.finalize()
    return nc


def _get_nc():
    if "nc" not in _cache:
        _cache["nc"] = build_kernel()
    return _cache["nc"]


def kernel(x, wq, wk, wv, wo):
    from concourse.bass_utils import run_bass_kernel_spmd

    x = np.asarray(x, dtype=np.float32)
    shards, c2, s2n = _host_prep(
        np.asarray(wq, np.float32), np.asarray(wk, np.float32),
        np.asarray(wv, np.float32), np.asarray(wo, np.float32))

    ones = np.ones((128, 128), dtype=np.float16)
    xt16 = [np.ascontiguousarray(x[b].T.astype(np.float16)) for b in range(B)]

    in_maps = []
    for c in range(8):
        b = c // 4
        m = dict(shards[c])
        m.update(xt16=xt16[b], c2=c2, s2n=s2n, ones=ones)
        in_maps.append(m)

    nc = _get_nc()
    res = run_bass_kernel_spmd(
        nc, in_maps, core_ids=list(range(8)),
        trace=bool(int(os.environ.get("KERNEL_TRACE", "0"))),
    )
    _cache["last_result"] = res
    parts = [r["out"].astype(np.float32) for r in res.results]
    out = np.empty((B, S, D), dtype=np.float32)
    for b in range(B):
        out[b] = parts[4 * b] + parts[4 * b + 1] + parts[4 * b + 2] + parts[4 * b + 3]
    return out


# revision 61
# speedup vs baseline: 1.0387x; 1.0007x over previous
"""Trainium2 Bass kernel for a local-attention layer (GQA + RoPE + banded mask).

Full computation (reference semantics, f32):
  q = x@wq, k = x@wk, v = x@wv  (B=2, S=2048, D=2048, Hq=16, Hkv=4, hd=128)
  rope(q), rope(k) interleaved-pair style
  banded causal attention, window=1024, softmax
  out = (probs @ v_rep) @ wo

Sharding: 8 cores = (batch b in {0,1}) x (kv-group g in {0..3}).
Core c handles batch c//4, kv head c%4 and its 4 q heads.  Each core
computes a partial (2048, 2048) f16 output (its heads' contribution
through wo rows); host sums the 4 partials per batch in f32.

Per-core kernel strategy (all matmuls fp16, f32 PSUM accum):
  - X^T pre-transposed on the host (f16) and plain-DMA'd into SBUF --
    no PE transposes, no PSUM staging.
  - Projections computed transposed one head at a time: QT/KT/VT =
    W^T X^T from natural W layout (lhsT=W chunk, rhs=XT chunk).
  - RoPE in "half-split" form on f16 SBUF tiles (host pre-permutes
    wq/wk columns; 1/sqrt(hd) folded into wq): y = x*c2 + swap(x)*s2n
    with duplicated cos and sign-folded sin so every two-input DVE op
    keeps both inputs at base partition 0 (HW verifier rule).
  - V transposed to natural (s, d) tiles by SBUF->SBUF DMA-transpose.
  - Scores transposed ST[j,i] = kt_tile^T @ QT, banded tight: per j-tile
    only the i-subtiles inside the band are computed; exp on ScalarE per
    written window.  Post-exp masks (gpsimd.affine_select, fill 0) only
    on the diagonal and window-edge 128x128 blocks.
  - pv/dn PSUM banks explicitly zeroed (ACT memzero / DVE memset), all
    accumulating matmuls start=False with skip_group_check so ragged
    band windows can accumulate in pipeline order.
  - Softmax denominator via ones[128,128] stationary matmul -> PSUM is
    the broadcast denominator; reciprocal on DVE; normalization fused
    into the PSUM->SBUF copy of the attention output.
  - o-proj accumulates in PSUM, staged to f16 SBUF (ACT/DVE alternating)
    and DMA'd out as f16; host sums the 4 partials per batch.
  - Software pipelining: chunk c's attention/o-proj interleaved with
    chunk c+1's projections; PSUM statically fits 8 banks; startup DMAs
    ordered just-in-time across the SWDGE and two HWDGE queues.
"""

import os
import numpy as np

B, S, D = 2, 2048, 2048
NH, NKV, HD = 16, 4, 128
WINDOW = 1024
ROPE_THETA = 10000.0
HQ = NH // NKV          # q heads per core = 4
QD = HQ * HD            # 512
NK = D // 128           # 16 contraction chunks
CH = 512                # s-chunk size
NCH = S // CH           # 4 chunks
NSUB = CH // 128        # 4 s-subtiles per chunk

_cache = {}


def _host_prep(wq, wk, wv, wo):
    """Per-core f16 weight slices with rope permutation + scale folded in."""
    # de-interleave permutation: dev col j <- ref col (2j if j<64 else 2(j-64)+1)
    perm = np.empty(HD, dtype=np.int64)
    perm[:64] = np.arange(64) * 2
    perm[64:] = np.arange(64) * 2 + 1

    scale = 1.0 / np.sqrt(np.float32(HD))
    wq_p = (wq.reshape(D, NH, HD)[:, :, perm].reshape(D, NH * HD) * scale)
    wk_p = wk.reshape(D, NKV, HD)[:, :, perm].reshape(D, NKV * HD)

    inv_freq = 1.0 / (ROPE_THETA ** (np.arange(0, HD, 2, dtype=np.float32) / HD))
    t = np.arange(S, dtype=np.float32)
    ang = np.outer(t, inv_freq)             # (S, 64)
    cosT = np.cos(ang).T.astype(np.float16)  # (64, S)
    sinT = np.sin(ang).T.astype(np.float16)
    # duplicated cos; sign-folded sin: y = x*c2 + swap(x)*s2n, all ops
    # partition-base aligned (HW: two SBUF inputs must share base partition)
    c2 = np.ascontiguousarray(np.concatenate([cosT, cosT], axis=0))    # (128,S)
    s2n = np.ascontiguousarray(np.concatenate([-sinT, sinT], axis=0))

    shards = []
    for c in range(8):
        g = c % 4
        wkv = np.concatenate(
            [wk_p[:, g * HD:(g + 1) * HD], wv[:, g * HD:(g + 1) * HD]], axis=1)
        sh = dict(
            wkv=np.ascontiguousarray(wkv).astype(np.float16),
            wq01=np.ascontiguousarray(
                wq_p[:, g * QD:g * QD + 2 * HD]).astype(np.float16),
            wq23=np.ascontiguousarray(
                wq_p[:, g * QD + 2 * HD:(g + 1) * QD]).astype(np.float16),
            wo=np.ascontiguousarray(wo[g * QD:(g + 1) * QD, :]).astype(np.float16),
        )
        shards.append(sh)
    return shards, c2, s2n


def _jt_win(c, jt):
    """Allowed i-subtile window [lo, hi] (global isub idx) for j-tile jt in chunk c."""
    lo = max(4 * c, jt)
    hi = min(4 * c + 3, jt + WINDOW // 128)
    return lo, hi


def build_kernel():
    import concourse.bass as bass
    import concourse.mybir as mybir
    import concourse.tile as tile
    from concourse import bacc

    f16 = mybir.dt.float16
    f32 = mybir.dt.float32
    EXP = mybir.ActivationFunctionType.Exp
    GE = mybir.AluOpType.is_ge
    NW = WINDOW // 128      # 8

    nc = bacc.Bacc("TRN2", target_bir_lowering=False, debug=False, num_devices=8)

    xt16_d = nc.dram_tensor("xt16", [D, S], f16, kind="ExternalInput").ap()
    wkv_d = nc.dram_tensor("wkv", [D, 2 * HD], f16, kind="ExternalInput").ap()
    wq01_d = nc.dram_tensor("wq01", [D, 2 * HD], f16, kind="ExternalInput").ap()
    wq23_d = nc.dram_tensor("wq23", [D, 2 * HD], f16, kind="ExternalInput").ap()
    wo_d = nc.dram_tensor("wo", [QD, D], f16, kind="ExternalInput").ap()
    c_d = nc.dram_tensor("c2", [128, S], f16, kind="ExternalInput").ap()
    s_d = nc.dram_tensor("s2n", [128, S], f16, kind="ExternalInput").ap()
    one_d = nc.dram_tensor("ones", [128, 128], f16, kind="ExternalInput").ap()
    out_d = nc.dram_tensor("out", [S, D], f16, kind="ExternalOutput").ap()

    with tile.TileContext(nc) as tc:
        with (
            tc.tile_pool(name="persist", bufs=1) as pp,
            tc.tile_pool(name="xtpool", bufs=2) as xtp,
            tc.tile_pool(name="qtpool", bufs=8) as qtp,
            tc.tile_pool(name="ropet", bufs=2) as rtp,
            tc.tile_pool(name="ropeab", bufs=4) as rab,
            tc.tile_pool(name="vtpool", bufs=2) as vtp,
            tc.tile_pool(name="ptpool", bufs=8) as ptp,
            tc.tile_pool(name="atpool", bufs=6) as atp,
            tc.tile_pool(name="rcpool", bufs=2) as rcp_p,
            tc.tile_pool(name="obpool", bufs=3) as obp,
            tc.tile_pool(name="psProj", bufs=2, space="PSUM") as pjp,
            tc.tile_pool(name="psBig", bufs=2, space="PSUM") as stp,
            tc.tile_pool(name="psPV", bufs=1, space="PSUM") as pvp,
            tc.tile_pool(name="psDN", bufs=1, space="PSUM") as dnp,
        ):
            # ---- persistent SBUF tensors -------------------------------
            wkv_sb = pp.tile([128, NK * 2 * HD], f16, tag="wkv")  # [k][wk|wv]
            wq_sb = pp.tile([128, NK * QD], f16, tag="wq")        # [k][:, qd]
            wo_sb = pp.tile([128, HQ * D], f16, tag="wo")       # [h][:, e]
            c_sb = pp.tile([128, S], f16, tag="c")
            s_sb = pp.tile([128, S], f16, tag="s")
            one_sb = pp.tile([128, 128], f16, tag="one")
            kt_sb = pp.tile([128, S], f16, tag="kt")            # rope'd K^T
            v_sb = pp.tile([128, NK * 128], f16, tag="v")       # [jt][s, d]

            xts = [None] * NCH

            def issue_xt(c, nsub=2):
                """Load host-pre-transposed X^T k-tiles for chunk c,
                split so early k-tiles land sooner."""
                xt = xtp.tile([128, NK * CH], f16, tag="xt", name=f"xt{c}")
                kn = NK // nsub
                for q in range(nsub):
                    nc.sync.dma_start(
                        xt[:, kn * q * CH:kn * (q + 1) * CH]
                        .rearrange("p (k s) -> p k s", k=kn),
                        xt16_d[kn * q * 128:kn * (q + 1) * 128,
                               c * CH:(c + 1) * CH]
                        .rearrange("(k p) s -> p k s", p=128))
                xts[c] = xt

            # startup DMA order matters: the DMA transfers serialize on a
            # shared device in arrival order, so issue just-in-time for
            # the chunk-0 pipeline (interleaved with the proj passes below).
            def dma_startup_head():
                # wkv via SWDGE (parallel with HWDGE), split in two so
                # the first k-tiles land sooner; x^T(0) quarters alternate
                # across the two HWDGE queues
                for q in range(4):
                    w0 = q * (NK // 4) * 2 * HD
                    r0 = q * (D // 4)
                    nc.gpsimd.dma_start(
                        wkv_sb[:, w0:w0 + (NK // 4) * 2 * HD]
                        .rearrange("p (k n) -> p k n", k=NK // 4),
                        wkv_d[r0:r0 + D // 4, :]
                        .rearrange("(k p) n -> p k n", p=128))
                xt = xtp.tile([128, NK * CH], f16, tag="xt", name="xt0")
                for q in range(4):
                    eng = nc.sync if q % 2 == 0 else nc.scalar
                    eng.dma_start(
                        xt[:, 4 * q * CH:4 * (q + 1) * CH]
                        .rearrange("p (k s) -> p k s", k=4),
                        xt16_d[4 * q * 128:4 * (q + 1) * 128, 0:CH]
                        .rearrange("(k p) s -> p k s", p=128))
                xts[0] = xt
                # trig + ones via SWDGE: lands early, in parallel with the
                # HWDGE stream (the rope ops head the DVE queue and would
                # otherwise stall all DVE retirement on these)
                nc.gpsimd.dma_start(c_sb[:], c_d)
                nc.gpsimd.dma_start(s_sb[:], s_d)
                nc.gpsimd.dma_start(one_sb[:], one_d)
                nc.scalar.dma_start(
                    wq_sb[:, 0:NK * 2 * HD]
                    .rearrange("p (k n) -> p k n", k=NK),
                    wq01_d.rearrange("(k p) n -> p k n", p=128))
                nc.scalar.dma_start(
                    wq_sb[:, NK * 2 * HD:]
                    .rearrange("p (k n) -> p k n", k=NK),
                    wq23_d.rearrange("(k p) n -> p k n", p=128))

            def rope(dst, src, c):
                """src (128, CH) f16 sbuf -> dst (128, CH) f16, half-split rope:
                dst = src*c2 + swap(src)*s2n.  Two-input DVE ops keep both
                inputs at base partition 0 (HW verifier requirement)."""
                cs = c_sb[:, c * CH:(c + 1) * CH]
                sn = s_sb[:, c * CH:(c + 1) * CH]
                sw = rab.tile([128, CH], f16, tag="ra")
                nc.vector.tensor_copy(sw[0:64, :], src[64:128, :])
                nc.vector.tensor_copy(sw[64:128, :], src[0:64, :])
                a = rab.tile([128, CH], f16, tag="ra")
                nc.vector.tensor_mul(a[:], src[:], cs)
                nc.vector.tensor_mul(sw[:], sw[:], sn)
                nc.vector.tensor_add(dst[:], a[:], sw[:])

            def proj_pass_off(c, w_sb, off, stride):
                """One 128-wide projection: returns f16 copy in SBUF."""
                ps = pjp.tile([128, CH], f32, tag="pj", name=f"pj{c}_{off}")
                xt = xts[c]
                for k in range(NK):
                    nc.tensor.matmul(
                        ps[:],
                        w_sb[:, k * stride + off: k * stride + off + 128],
                        xt[:, k * CH:(k + 1) * CH],
                        start=(k == 0), stop=(k == NK - 1),
                    )
                xf = rtp.tile([128, CH], f16, tag="xf", name=f"xf{c}_{off}")
                nc.vector.tensor_copy(xf[:], ps[:])
                return xf

            proj_pass = proj_pass_off

            def emit_proj(c, which):
                """which in {'k','v','q0'..'q3'}"""
                if which == "k":
                    xf = proj_pass_off(c, wkv_sb, 0, 2 * HD)
                    rope(kt_sb[:, c * CH:(c + 1) * CH], xf, c)
                elif which == "v":
                    ps = pjp.tile([128, CH], f32, tag="pj", name=f"pjv{c}")
                    xt = xts[c]
                    for k in range(NK):
                        nc.tensor.matmul(
                            ps[:], wkv_sb[:, k * 2 * HD + HD: (k + 1) * 2 * HD],
                            xt[:, k * CH:(k + 1) * CH],
                            start=(k == 0), stop=(k == NK - 1),
                        )
                    vt = vtp.tile([128, CH], f16, tag="vt", name=f"vt{c}")
                    nc.scalar.copy(vt[:], ps[:])
                    # natural (s, d) tiles via SBUF->SBUF DMA transpose
                    nc.sync.dma_start_transpose(
                        v_sb[:, c * NSUB * 128:(c + 1) * NSUB * 128]
                        .rearrange("p (g d) -> p g d", g=NSUB),
                        vt[:])
                else:
                    h = int(which[1])
                    off = (h // 2) * NK * 2 * HD
                    xf = proj_pass_off(c, wq_sb, off + (h % 2) * HD, 2 * HD)
                    qt = qtp.tile([128, CH], f16, tag="qt", name=f"qt{c}_{h}")
                    rope(qt, xf, c)
                    qts[(c, h)] = qt

            qts = {}
            ats = {}

            def emit_att_head(c, h, alt_pool=False):
                jt0 = max(0, 4 * c - NW)
                jts = list(range(jt0, 4 * c + 4))
                # tail chunk: odd heads borrow the (idle) projection PSUM
                # pool so head boundaries double-buffer pv/dn
                pool_v = pjp if alt_pool else pvp
                pool_d = pjp if alt_pool else dnp
                pv_ps = pool_v.tile([128, CH], f32, tag="pj" if alt_pool else "pv",
                                    name=f"pv{c}_{h}")
                dn_ps = pool_d.tile([128, CH], f32, tag="pj" if alt_pool else "dn",
                                    name=f"dn{c}_{h}")
                # explicit zero + start=False accumulation everywhere: lets
                # ragged-window matmuls accumulate in natural (pipeline) order
                nc.vector.memset(pv_ps[:], 0.0)
                nc.vector.memset(dn_ps[:], 0.0)
                pts = []
                for pi in range(0, len(jts), 2):
                    pair = jts[pi:pi + 2]
                    st = stp.tile([128, 2 * CH], f32, tag="big", name=f"st{c}_{h}_{pi}")
                    for q_, jt in enumerate(pair):
                        lo, hi = _jt_win(c, jt)
                        a0 = (lo - 4 * c) * 128
                        a1 = (hi - 4 * c + 1) * 128
                        nc.tensor.matmul(
                            st[:, q_ * CH + a0: q_ * CH + a1],
                            kt_sb[:, jt * 128:(jt + 1) * 128],
                            qts[(c, h)][:, a0:a1],
                            start=True, stop=True,
                        )
                    pt = ptp.tile([128, 2 * CH], f16, tag="pt", name=f"pt{c}_{h}_{pi}")
                    for q_, jt in enumerate(pair):
                        lo, hi = _jt_win(c, jt)
                        a0 = q_ * CH + (lo - 4 * c) * 128
                        a1 = q_ * CH + (hi - 4 * c + 1) * 128
                        nc.scalar.activation(pt[:, a0:a1], st[:, a0:a1], EXP)
                    for q_, jt in enumerate(pair):
                        if 4 * c <= jt:          # diagonal block: keep j <= i
                            o = q_ * CH + (jt - 4 * c) * 128
                            nc.gpsimd.affine_select(
                                out=pt[:, o:o + 128], in_=pt[:, o:o + 128],
                                pattern=[[1, 128]], compare_op=GE, fill=0.0,
                                base=0, channel_multiplier=-1,
                            )
                        if jt + NW <= 4 * c + 3:  # window edge: keep j > i-W-1
                            o = q_ * CH + (jt + NW - 4 * c) * 128
                            nc.gpsimd.affine_select(
                                out=pt[:, o:o + 128], in_=pt[:, o:o + 128],
                                pattern=[[-1, 128]], compare_op=GE, fill=0.0,
                                base=0, channel_multiplier=1,
                            )
                    pts.append((pair, pt))

                items = []
                for pi, (pair, pt) in enumerate(pts):
                    for q_, jt in enumerate(pair):
                        lo, hi = _jt_win(c, jt)
                        items.append((jt, (lo - 4 * c) * 128,
                                      (hi - 4 * c + 1) * 128, q_, pt))
                for n_, (jt, a0, a1, q_, pt) in enumerate(items):
                    nc.tensor.matmul(
                        pv_ps[:, a0:a1],
                        v_sb[:, jt * 128:(jt + 1) * 128],
                        pt[:, q_ * CH + a0: q_ * CH + a1],
                        start=False, stop=(n_ == len(items) - 1),
                        skip_group_check=True,
                    )
                for n_, (jt, a0, a1, q_, pt) in enumerate(items):
                    nc.tensor.matmul(
                        dn_ps[:, a0:a1],
                        one_sb[:],
                        pt[:, q_ * CH + a0: q_ * CH + a1],
                        start=False, stop=(n_ == len(items) - 1),
                        skip_group_check=True,
                    )
                rcp = rcp_p.tile([128, CH], f32, tag="rcp", name=f"rcp{c}_{h}")
                nc.vector.reciprocal(rcp[:], dn_ps[:])
                at = atp.tile([128, CH], f16, tag="at", name=f"at{c}_{h}")
                nc.vector.tensor_mul(at[:], pv_ps[:], rcp[:])
                ats[(c, h)] = at

            def emit_oproj(c):
                for g in range(NSUB):
                    for half in range(2):
                        op_ = stp.tile([128, 2 * CH], f32, tag="big",
                                       name=f"op{c}_{g}_{half}")
                        for e2 in range(2):
                            ecol = half * 1024 + e2 * CH
                            for h in range(HQ):
                                nc.tensor.matmul(
                                    op_[:, e2 * CH:(e2 + 1) * CH],
                                    ats[(c, h)][:, g * 128:(g + 1) * 128],
                                    wo_sb[:, h * D + ecol: h * D + ecol + CH],
                                    start=(h == 0), stop=(h == HQ - 1),
                                )
                        ob = obp.tile([128, 2 * CH], f16, tag="ob",
                                      name=f"ob{c}_{g}_{half}")
                        if half == 0 or g == NSUB - 1:
                            # last group's copies on ACT too: it is idle at
                            # the block boundary and the st/op PSUM slots it
                            # frees gate the next chunk's first scores
                            nc.scalar.copy(ob[:], op_[:])
                        else:
                            nc.vector.tensor_copy(ob[:], op_[:])
                        nc.sync.dma_start(
                            out_d[c * CH + g * 128: c * CH + (g + 1) * 128,
                                  half * 1024:(half + 1) * 1024], ob[:])

            # ---- pipeline ----------------------------------------------
            dma_startup_head()
            emit_proj(0, "k")
            emit_proj(0, "v")
            issue_xt(1)
            emit_proj(0, "q0")
            emit_proj(0, "q1")
            nc.sync.dma_start(
                wo_sb[:].rearrange("p (h n) -> p h n", h=HQ),
                wo_d.rearrange("(h p) n -> p h n", p=128))
            emit_proj(0, "q2")
            emit_proj(0, "q3")
            for c in range(1, NCH + 1):
                if c < NCH:
                    # interleave chunk c-1 attention with chunk c projections
                    emit_att_head(c - 1, 0)
                    emit_proj(c, "k")
                    emit_att_head(c - 1, 1)
                    emit_proj(c, "v")
                    emit_att_head(c - 1, 2)
                    emit_proj(c, "q0")
                    emit_att_head(c - 1, 3)
                    emit_proj(c, "q1")
                    emit_oproj(c - 1)
                    if c + 1 < NCH:
                        issue_xt(c + 1)
                    emit_proj(c, "q2")
                    emit_proj(c, "q3")
                else:
                    for h in range(HQ):
                        emit_att_head(c - 1, h, alt_pool=(h % 2 == 1))
                    emit_oproj(c - 1)
    nc.finalize()
    return nc


def _get_nc():
    if "nc" not in _cache:
        _cache["nc"] = build_kernel()
    return _cache["nc"]


def kernel(x, wq, wk, wv, wo):
    from concourse.bass_utils import run_bass_kernel_spmd

    x = np.asarray(x, dtype=np.float32)
    shards, c2, s2n = _host_prep(
        np.asarray(wq, np.float32), np.asarray(wk, np.float32),
        np.asarray(wv, np.float32), np.asarray(wo, np.float32))

    ones = np.ones((128, 128), dtype=np.float16)
    xt16 = [np.ascontiguousarray(x[b].T.astype(np.float16)) for b in range(B)]

    in_maps = []
    for c in range(8):
        b = c // 4
        m = dict(shards[c])
        m.update(xt16=xt16[b], c2=c2, s2n=s2n, ones=ones)
        in_maps.append(m)

    nc = _get_nc()
    res = run_bass_kernel_spmd(
        nc, in_maps, core_ids=list(range(8)),
        trace=bool(int(os.environ.get("KERNEL_TRACE", "0"))),
    )
    _cache["last_result"] = res
    parts = [r["out"].astype(np.float32) for r in res.results]
    out = np.empty((B, S, D), dtype=np.float32)
    for b in range(B):
        out[b] = parts[4 * b] + parts[4 * b + 1] + parts[4 * b + 2] + parts[4 * b + 3]
    return out
